# revision 66
# baseline (speedup 1.0000x reference)
# SAGAN self-attention (B=4, H=W=64, C=64, D=8) on 8 TRN2 NeuronCores.
#
# Sharding: core i = (batch b=i//2, half h=i%2). Each core computes rows
# [h*2048, (h+1)*2048) of the 4096x4096 attention for its batch, fully fused
# in SBUF (no NxN matrix ever touches HBM, no collectives).
#
# Changes over the padded-K baseline (93us -> ~83us raw, ~77us at the fast
# chip clock; raw runs vary +-20% with a global clock state -- normalize by
# the 1536-col ACTIVATE duration, fast ~1433ns):
# - Scores mostly run as 2x2 grids of K=64 [64ch x 64keys x 512q] matmuls on
#   disjoint PE-array quadrants (channels duplicated on partitions 0-63 and
#   64-127 of XT2/GP2): two key chunks stream concurrently per ~512 cycles.
#   HAM (the PE clock gate) counts only full 128x128-array activity as
#   "busy", so every FULL_EVERY-th chunk is one full-array K=128 zero-padded
#   matmul (stationary XT0) as a heartbeat that holds 2.4 GHz; n-tile 0's
#   first swaths run all-full on the duplicated rows (scores doubled, fixed
#   by exp scale 0.5) to warm the clock through the startup ramp.
# - The per-key score bias d_m = bg.(f_m+bf) is folded MULTIPLICATIVELY into
#   the PV stationary: hv' = e^d * hv and the denominator ones-column is e^d,
#   so exp engines compute plain exp(s) and the bias costs nothing.
# - PV matmuls are col-tiled 4x: the [keys, 32] stationaries of 4 chunks go
#   to the four 32-col strips (tile_position (0, 32c), c = chunk%4), all
#   accumulating into one psum bank psv4[128, 512]; they issue in lagged
#   pairs of swath-groups so 5-6 run back-to-back (Dstart~0 measured) and
#   the PE queue always has score work while the exp engines run. The strip
#   partial sums are absorbed free by the epilogue's K=128 contraction (WVE
#   carries Wv rows at partitions {1-8,33-40,65-72,97-104} plus an E8
#   one-hot column at {0,32,64,96} -- strip row 0 is the denominator since
#   engine APs must start 32-aligned -- one matmul per query block yields
#   o_un AND the summed denominator).
# - exp is SPLIT between ScalarE (ACT Exp LUT, 1 elem/lane/cyc) and a custom
#   VectorE op EXP16 (1 elem/lane/cyc): (1 + z/16 + z^2/512)^16 =
#   exp(z)*e^(-z^3/1536+..) -- 8 ALU stages, pure fp32. Rel err ~1e-1 at
#   |z|=5 tails washes to ~2e-6 at the output through softmax averaging.
#   Swaths alternate engines (24 ACT / 20 DVE) so both exp concurrently at
#   a combined ~276 G elem/s; psum double-buffering makes the steady state
#   exp-latency-bound (measured; a bufs=3 2-chunk-swath variant loses to
#   per-call overhead).
# - Output DMA is contiguous [128, 16*C]; the host unpermutes. The final
#   n-tile's epilogue is split in halves to shorten the drain tail.
#
# Host precomputes the tiny projections (0.6% of FLOPs) and folds gamma into
# Wv and gamma*(bh@Wv+bv) into the residual. Attention matmuls use bf16
# operands; PSUM accumulation is fp32.

import numpy as np
import ml_dtypes

import concourse.bacc as bacc
import concourse.tile as tile
import concourse.mybir as mybir
from concourse.alu_op_type import AluOpType
from concourse.bass_utils import run_bass_kernel_spmd

F32 = mybir.dt.float32
BF16 = mybir.dt.bfloat16
AFT = mybir.ActivationFunctionType

B, HH, WW, C = 4, 64, 64, 64
N = HH * WW          # 4096 sequence positions per batch
D = 8                # qkv channel dim
RPC = N // 2         # rows per core (2048)
NCORES = 8
MC = N // 128        # 32 key chunks of 128
NT = 4               # n-tiles of 512 query rows each
TN = 512
# THREE psum score slots of [3,2,2] chunks (6144+4096+4096 bytes; psum is
# byte-granular and the epilogue tile time-shares the psv bank): exp(s)
# frees its slot two swaths before the PE needs it again, so the score
# stream no longer serializes behind in-flight exps (with two slots the
# steady state was chain-paced at exp+scores+handoff per 2 swaths)
SW = [3, 2, 2] * 4 + [2, 2]
# exp engine assignment: 3-chunk slots split across BOTH engines (shortest
# chain latency); 2-chunk slots alternate DVE (si%3==1) / ACT (si%3==2)
# chunks computed as one full-array K=128 zero-padded matmul instead of a
# K=64 2x2 grid pair: one full matmul every ~1.7us keeps the HAM clock gate
# at 2.4 GHz (partial-array matmuls alone read as "idle" and it re-throttles;
# FULL_EVERY=6 measured marginal against the 3.4us MID window -- roughly half
# of runs re-throttled mid-kernel and ran the PE cold)
FULL_EVERY = 4
FULL_CHUNKS = tuple(range(0, MC, FULL_EVERY))


def _register_exp16():
    """Custom DVE op: out = (1 + z/16 + (z/16)^2/2)^16 ~ exp(z).

    Registered via the documented dve_ops extension path (append to OPS);
    done at import time here because the concourse tree is read-only.
    8 ALU stages: mul, add, mul, add, 4x square -> 1 elem/lane/cycle.
    """
    import concourse.dve_ops as dve_ops
    from concourse.dve_spec import Spec, Src0, One, C1, C2, sq

    if "EXP16_SAGAN" in dve_ops._SUB_OPCODE_FOR_NAME:
        return next(o for o in dve_ops.OPS if o.name == "EXP16_SAGAN")

    def _ref(in0, in1, s0, s1, imm2):
        z = in0.astype(np.float32)
        q = (1.0 + z * np.float32(imm2) + (z * np.float32(imm2)) ** 2 / 2.0)
        return (q ** 16).astype(np.float32)

    t2 = Src0 * C1 + C2
    q = Src0 * t2 + One
    op = dve_ops.DveOp(
        "EXP16_SAGAN",
        Spec(body=sq(sq(sq(sq(q)))), reference=_ref),
        subdim=False,
        uops_sha={"v3": "b6b488f5e7e070ba", "v4": "cea1bf697c4845eb"},
    )
    dve_ops.OPS.append(op)
    dve_ops.CUSTOM_DVE_SPECS[op.name] = op.spec
    dve_ops._SUB_OPCODE_FOR_NAME[op.name] = (
        dve_ops._CUSTOM_DVE_ROW_BASE + len(dve_ops.OPS) - 1
    )
    return op


EXP16_OP = _register_exp16()


def _build():
    nc = bacc.Bacc("TRN2", target_bir_lowering=False, debug=False,
                   num_devices=NCORES)

    xt2 = nc.dram_tensor("xt2", [128, N], BF16, kind="ExternalInput").ap()
    xt0 = nc.dram_tensor("xt0", [128, len(FULL_CHUNKS) * 128], BF16,
                         kind="ExternalInput").ap()
    gp2 = nc.dram_tensor("gp2", [128, RPC], BF16, kind="ExternalInput").ap()
    hvo = nc.dram_tensor("hvo", [128, MC * 32], BF16,
                         kind="ExternalInput").ap()
    xrp = nc.dram_tensor("xrp", [128, RPC // 128 * C], F32,
                         kind="ExternalInput").ap()
    wv2 = nc.dram_tensor("wv2", [128, C + 1], BF16, kind="ExternalInput").ap()
    out = nc.dram_tensor("out", [128, RPC // 128 * C], F32,
                         kind="ExternalOutput").ap()

    with tile.TileContext(nc) as tc:
        with tc.tile_pool(name="const", bufs=1) as const:
            XT2 = const.tile([128, N], BF16)
            XT0 = const.tile([128, len(FULL_CHUNKS) * 128], BF16)
            GP2 = const.tile([128, RPC], BF16)
            HVO = const.tile([128, MC * 32], BF16)
            XRP = const.tile([128, RPC // 128 * C], F32)
            WVE = const.tile([128, C + 1], BF16)
            PRE = const.tile([1, 1], F32)
            PRE2 = const.tile([1, 4], BF16)
            WUP = const.tile([128, 256], BF16)

            # input DMAs in first-use order: swath 0 of n-tile 0 needs
            # GP2 cols 0:512, XT2 cols 0:384, HVO cols 0:96
            nc.sync.dma_start(GP2[:, 0:512], gp2[:, 0:512])
            nc.sync.dma_start(XT2[:, 0:512], xt2[:, 0:512])
            nc.sync.dma_start(HVO[:, 0:256], hvo[:, 0:256])
            nc.sync.dma_start(XT2[:, 512:2048], xt2[:, 512:2048])
            nc.sync.dma_start(HVO[:, 256:1024], hvo[:, 256:1024])
            nc.sync.dma_start(XT2[:, 2048:4096], xt2[:, 2048:4096])
            nc.sync.dma_start(XT0[:], xt0[:])
            nc.sync.dma_start(GP2[:, 512:2048], gp2[:, 512:2048])
            nc.sync.dma_start(WVE[:], wv2[:])
            nc.sync.dma_start(XRP[:], xrp[:])
            nc.vector.memset(WUP[:], 0.0)
            # dummy exp: hoists the one-time ACT table load (~2.7us) into the
            # initial DMA wait instead of the first swath's critical path
            nc.scalar.activation(PRE[:], WUP[0:1, 0:1], AFT.Exp)
            # dummy EXP16: flushes any lazy DVE custom-table init
            nc.vector._custom_dve(EXP16_OP, out=PRE2[:, 0:1],
                                  in0=WUP[0:1, 0:1],
                                  s1=1.0 / 512, imm2=1.0 / 16)

            with tc.tile_pool(name="ps_a", bufs=1, space="PSUM") as ps_a, \
                 tc.tile_pool(name="ps_b", bufs=1, space="PSUM") as ps_b, \
                 tc.tile_pool(name="ps_c", bufs=1, space="PSUM") as ps_c, \
                 tc.tile_pool(name="ps_v", bufs=1, space="PSUM") as ps_vp, \
                 tc.tile_pool(name="expp", bufs=8) as expp, \
                 tc.tile_pool(name="vd2p", bufs=2) as vd2p, \
                 tc.tile_pool(name="scolp", bufs=2) as scolp, \
                 tc.tile_pool(name="osbp", bufs=2) as osbp:
                # PE warm-up: ~4us of K=128 matmuls during the initial DMA
                # wait lifts the HAM clock throttle (1.2 -> 2.4 GHz) before
                # the real swaths begin; output is scratch, never read
                wps = ps_a.tile([128, 1536], F32, tag="ps")
                for wi in range(14):
                    nc.tensor.matmul(wps[:, 0:256], lhsT=WUP[:, 0:128],
                                     rhs=WUP[:], start=True, stop=True,
                                     skip_group_check=True)

                def epilogue(nt, vd2, split=False):
                    # one [128,65] rhs per query block: cols 0-63 -> o_un,
                    # col 64 (E8 one-hots) -> summed denominator. split=True
                    # (final n-tile) pipelines two halves to shorten the
                    # tail. The tile time-shares the psv bank: psv(nt) was
                    # fully read (vd2 copy) before this, and psv(nt+1) is
                    # allocated lazily after this tile's STT reads finish
                    pse_t = ps_vp.tile([128, 512], F32, tag="psv")
                    pse = pse_t[:, 0:260]
                    scol = scolp.tile([128, 4], F32)
                    osb4 = osbp.tile([128, 4 * C], F32)
                    for h in range(2) if split else (0,):
                        nbs = (2 * h, 2 * h + 1) if split else range(4)
                        for nb in nbs:
                            nc.tensor.matmul(
                                pse[:, nb * 65:(nb + 1) * 65],
                                lhsT=vd2[:, nb * 128:(nb + 1) * 128],
                                rhs=WVE[:], start=True, stop=True)
                        if split:
                            nc.vector.reciprocal(
                                scol[:, 2 * h:2 * h + 2],
                                pse[:, 130 * h + 64:130 * h + 130:65])
                        else:
                            nc.vector.reciprocal(scol[:], pse[:, 64:260:65])
                        for nb in nbs:
                            t = nt * 4 + nb
                            nc.vector.scalar_tensor_tensor(
                                osb4[:, nb * C:(nb + 1) * C],
                                pse[:, nb * 65:nb * 65 + 64],
                                scol[:, nb:nb + 1],
                                XRP[:, t * C:(t + 1) * C],
                                op0=AluOpType.mult, op1=AluOpType.add)
                        if split:
                            o0 = nt * 4 * C + h * 2 * C
                            nc.sync.dma_start(out[:, o0:o0 + 2 * C],
                                              osb4[:, h * 2 * C:(h + 1) * 2 * C])
                    if not split:
                        nc.sync.dma_start(out[:, nt * 4 * C:(nt + 1) * 4 * C],
                                          osb4[:])

                def emit_pv(psv, ex, m0, sw):
                    for k in range(sw):
                        mm = m0 + k
                        c4 = (mm % 4) * 32
                        nc.tensor.matmul(
                            psv[c4:c4 + 32, :],
                            lhsT=HVO[:, mm * 32:(mm + 1) * 32],
                            rhs=ex[:, k * 512:(k + 1) * 512],
                            start=(mm < 4), stop=(mm >= MC - 4),
                            skip_group_check=True,
                            tile_position=(0, c4))

                pending = None
                for nt in range(NT):
                    n0 = nt * TN
                    psv = None   # allocated lazily AFTER the epilogue's pse
                    m = 0        # so the shared ps_v pool rotates in order
                    prev_pv = []
                    for si, sw in enumerate(SW):
                        pool = (ps_a, ps_b, ps_c)[si % 3]
                        ps = pool.tile([128, 1536 if si % 3 == 0 else 1024],
                                       F32)
                        ex = expp.tile([128, 1536], BF16)
                        w = sw * 512
                        # the first swaths run every chunk as a full-array
                        # K=128 matmul on the duplicated channel rows (scores
                        # come out doubled; the exp scale of 0.5 corrects) so
                        # the HAM clock gate warms through the startup ramp.
                        # Everything after uses K=64 grid pairs with a
                        # full-array XT0 matmul every FULL_EVERY chunks as
                        # heartbeat.
                        dup = nt == 0 and si < 6
                        for k in range(sw):
                            mm = m + k
                            if dup or mm % FULL_EVERY == 0:
                                if dup:
                                    lw = XT2[:, mm * 128:(mm + 1) * 128]
                                else:
                                    fi = mm // FULL_EVERY
                                    lw = XT0[:, fi * 128:(fi + 1) * 128]
                                nc.tensor.matmul(
                                    ps[:, k * 512:(k + 1) * 512],
                                    lhsT=lw, rhs=GP2[:, n0:n0 + TN],
                                    start=True, stop=True)
                                continue
                            cA = mm * 128
                            # K=64 grid pair on disjoint array quadrants;
                            # parity alternates the channel-copy row strips
                            rl = 0 if mm % 2 == 0 else 64
                            rh = 64 - rl
                            nc.tensor.matmul(
                                ps[0:64, k * 512:(k + 1) * 512],
                                lhsT=XT2[rl:rl + 64, cA:cA + 64],
                                rhs=GP2[rl:rl + 64, n0:n0 + TN],
                                start=True, stop=True)
                            nc.tensor.matmul(
                                ps[64:128, k * 512:(k + 1) * 512],
                                lhsT=XT2[rh:rh + 64, cA + 64:cA + 128],
                                rhs=GP2[rh:rh + 64, n0:n0 + TN],
                                start=True, stop=True)
                        sc = 0.5 if dup else 1.0
                        if sw == 3:
                            # the 3-chunk slot is the longest chain link:
                            # split its exp across BOTH engines so the slot
                            # frees after ~1.1us instead of ~1.6us
                            nc.scalar.activation(ex[:, 0:1024],
                                                 ps[:, 0:1024],
                                                 AFT.Exp, scale=sc)
                            nc.vector._custom_dve(
                                EXP16_OP, out=ex[:, 1024:1536],
                                in0=ps[:, 1024:1536],
                                s1=sc * sc / 512, imm2=sc / 16)
                        elif si % 3 == 1:
                            nc.vector._custom_dve(
                                EXP16_OP, out=ex[:, :w], in0=ps[:, :w],
                                s1=sc * sc / 512, imm2=sc / 16)
                        else:
                            nc.scalar.activation(ex[:, :w], ps[:, :w],
                                                 AFT.Exp, scale=sc)
                        # PV is emitted one-to-two swaths late in PAIRS of
                        # swath groups: the PE queue always has score matmuls
                        # during exp, and 5-6 back-to-back col-tiled PV
                        # matmuls amortize one fill+drain (same-strip psum
                        # accumulates are hw-serialized in order and commute)
                        prev_pv.append((ex, m, sw))
                        if len(prev_pv) == 3:
                            if psv is None:
                                psv = ps_vp.tile([128, TN], F32,
                                                 tag="psv")
                            for args in prev_pv[:2]:
                                emit_pv(psv, *args)
                            prev_pv = prev_pv[2:]
                        m += sw
                        # previous n-tile's epilogue in swath 0: its DVE work
                        # (recip+STT) lands in the DVE queue during the
                        # n-tile-boundary refill slack instead of between
                        # chain-critical exp calls
                        if si == 0 and pending is not None:
                            epilogue(*pending)
                            pending = None
                    for args in prev_pv:
                        emit_pv(psv, *args)
                    vd2 = vd2p.tile([128, TN], BF16)
                    # cast on the scalar engine: the vector engine carries
                    # the bigger share of exp + the whole epilogue tail
                    if nt == NT - 1:   # split so the tail epilogue's first
                        nc.scalar.copy(vd2[:, 0:256], psv[:, 0:256])
                        nc.scalar.copy(vd2[:, 256:512], psv[:, 256:512])
                    else:
                        nc.scalar.copy(vd2[:, :], psv[:, :])
                    pending = (nt, vd2)
                epilogue(*pending, split=True)
    nc.compile()
    return nc


_CACHE = {}


def _get_compiled():
    if "nc" not in _CACHE:
        _CACHE["nc"] = _build()
    return _CACHE["nc"]


def _make_in_maps(x, Wf, bf, Wg, bg, Wh, bh, Wv, bv, gamma):
    x = np.asarray(x, np.float32)
    Wf = np.asarray(Wf, np.float32)
    Wg = np.asarray(Wg, np.float32)
    Wh = np.asarray(Wh, np.float32)
    Wv = np.asarray(Wv, np.float32)
    bf = np.asarray(bf, np.float32)
    bg = np.asarray(bg, np.float32)
    bh = np.asarray(bh, np.float32)
    bv = np.asarray(bv, np.float32)
    g0 = float(np.asarray(gamma, np.float32).reshape(-1)[0])

    xf = x.reshape(B, N, C)
    P = Wf @ Wg.T                            # [C, C] score kernel
    wfbg = Wf @ bg                           # [C] per-key bias direction
    bgbf = float(bg @ bf)
    res_bias = g0 * (bh @ Wv + bv)           # [C] folded into residual
    wv2 = np.zeros((128, C + 1), np.float32)
    for sb in range(4):                      # strip-summed by the epilogue MM
        wv2[sb * 32 + 1:sb * 32 + 1 + D, 0:C] = g0 * Wv
        wv2[sb * 32, C] = 1.0                # E8 one-hot -> denominator sum
    wv2 = wv2.astype(ml_dtypes.bfloat16)

    in_maps = []
    for i in range(NCORES):
        b, h = divmod(i, 2)
        r0 = h * RPC
        xt2 = np.empty((128, N), np.float32)
        xt2[0:C] = xf[b].T                   # channels, duplicated per half
        xt2[C:2 * C] = xf[b].T               # (for the K=64 grid pairs)
        gp2 = np.empty((128, RPC), np.float32)
        gp2[0:C] = P @ xf[b, r0:r0 + RPC].T
        gp2[C:2 * C] = gp2[0:C]
        xt0 = np.zeros((128, len(FULL_CHUNKS) * 128), np.float32)
        for fi, mm in enumerate(FULL_CHUNKS):  # zero rows 64-127: full-K
            xt0[0:C, fi * 128:(fi + 1) * 128] = (  # matmuls ignore the dup
                xf[b, mm * 128:(mm + 1) * 128].T)
        d_m = xf[b] @ wfbg + bgbf            # per-key score bias
        ed = np.exp(d_m)                     # folded multiplicatively
        hv = xf[b] @ Wh                      # [N, D] (bh folds into res_bias)
        ho = np.zeros((MC, 128, 32), np.float32)
        ho[:, :, 0] = ed.reshape(MC, 128)
        ho[:, :, 1:1 + D] = (ed[:, None] * hv).reshape(MC, 128, D)
        ho = np.ascontiguousarray(ho.transpose(1, 0, 2).reshape(128, -1))
        xr = xf[b, r0:r0 + RPC] + res_bias   # [RPC, C]
        xrp = np.ascontiguousarray(
            xr.reshape(RPC // 128, 128, C).transpose(1, 0, 2).reshape(128, -1))
        in_maps.append({"xt2": xt2.astype(ml_dtypes.bfloat16),
                        "xt0": xt0.astype(ml_dtypes.bfloat16),
                        "gp2": gp2.astype(ml_dtypes.bfloat16),
                        "hvo": ho.astype(ml_dtypes.bfloat16),
                        "xrp": xrp, "wv2": wv2})
    return in_maps


def _assemble(results):
    outf = np.empty((B, N, C), np.float32)
    for i in range(NCORES):
        b, h = divmod(i, 2)
        o = results[i]["out"].reshape(128, RPC // 128, C)
        outf[b, h * RPC:(h + 1) * RPC] = (
            o.transpose(1, 0, 2).reshape(RPC, C))
    return outf.reshape(B, HH, WW, C)


def run(inputs, **spmd_kwargs):
    """Returns (output, BassKernelResults)."""
    nc = _get_compiled()
    in_maps = _make_in_maps(**inputs)
    res = run_bass_kernel_spmd(nc, in_maps, core_ids=list(range(NCORES)),
                               **spmd_kwargs)
    return _assemble(res.results), res


def kernel(**inputs):
    out, _ = run(inputs)
    return out


# revision 70
# speedup vs baseline: 1.1220x; 1.1220x over previous
# SAGAN self-attention (B=4, H=W=64, C=64, D=8) on 8 TRN2 NeuronCores.
#
# Sharding: core i = (batch b=i//2, half h=i%2). Each core computes rows
# [h*2048, (h+1)*2048) of the 4096x4096 attention for its batch, fully fused
# in SBUF (no NxN matrix ever touches HBM, no collectives).
#
# Changes over the padded-K baseline (93us -> ~83us raw, ~77us at the fast
# chip clock; raw runs vary +-20% with a global clock state -- normalize by
# the 1536-col ACTIVATE duration, fast ~1433ns):
# - Scores mostly run as 2x2 grids of K=64 [64ch x 64keys x 512q] matmuls on
#   disjoint PE-array quadrants (channels duplicated on partitions 0-63 and
#   64-127 of XT2/GP2): two key chunks stream concurrently per ~512 cycles.
#   HAM (the PE clock gate) counts only full 128x128-array activity as
#   "busy", so every FULL_EVERY-th chunk is one full-array K=128 zero-padded
#   matmul (stationary XT0) as a heartbeat that holds 2.4 GHz; n-tile 0's
#   first swaths run all-full on the duplicated rows (scores doubled, fixed
#   by exp scale 0.5) to warm the clock through the startup ramp.
# - The per-key score bias d_m = bg.(f_m+bf) is folded MULTIPLICATIVELY into
#   the PV stationary: hv' = e^d * hv and the denominator ones-column is e^d,
#   so exp engines compute plain exp(s) and the bias costs nothing.
# - PV matmuls are col-tiled 4x: the [keys, 32] stationaries of 4 chunks go
#   to the four 32-col strips (tile_position (0, 32c), c = chunk%4), all
#   accumulating into one psum bank psv4[128, 512]; they issue in lagged
#   pairs of swath-groups so 5-6 run back-to-back (Dstart~0 measured) and
#   the PE queue always has score work while the exp engines run. The strip
#   partial sums are absorbed free by the epilogue's K=128 contraction (WVE
#   carries Wv rows at partitions {1-8,33-40,65-72,97-104} plus an E8
#   one-hot column at {0,32,64,96} -- strip row 0 is the denominator since
#   engine APs must start 32-aligned -- one matmul per query block yields
#   o_un AND the summed denominator).
# - exp is SPLIT between ScalarE (ACT Exp LUT, 1 elem/lane/cyc) and a custom
#   VectorE op EXP16 (1 elem/lane/cyc): (1 + z/16 + z^2/512)^16 =
#   exp(z)*e^(-z^3/1536+..) -- 8 ALU stages, pure fp32. Rel err ~1e-1 at
#   |z|=5 tails washes to ~2e-6 at the output through softmax averaging.
#   Swaths alternate engines (24 ACT / 20 DVE) so both exp concurrently at
#   a combined ~276 G elem/s; psum double-buffering makes the steady state
#   exp-latency-bound (measured; a bufs=3 2-chunk-swath variant loses to
#   per-call overhead).
# - Output DMA is contiguous [128, 16*C]; the host unpermutes. The final
#   n-tile's epilogue is split in halves to shorten the drain tail.
#
# Host precomputes the tiny projections (0.6% of FLOPs) and folds gamma into
# Wv and gamma*(bh@Wv+bv) into the residual. Attention matmuls use bf16
# operands; PSUM accumulation is fp32.

import numpy as np
import ml_dtypes

import concourse.bacc as bacc
import concourse.tile as tile
import concourse.mybir as mybir
from concourse.alu_op_type import AluOpType
from concourse.bass_utils import run_bass_kernel_spmd

F32 = mybir.dt.float32
BF16 = mybir.dt.bfloat16
AFT = mybir.ActivationFunctionType

B, HH, WW, C = 4, 64, 64, 64
N = HH * WW          # 4096 sequence positions per batch
D = 8                # qkv channel dim
RPC = N // 2         # rows per core (2048)
NCORES = 8
MC = N // 128        # 32 key chunks of 128
NT = 4               # n-tiles of 512 query rows each
TN = 512
# THREE psum score slots of [3,2,2] chunks (6144+4096+4096 bytes; psum is
# byte-granular and the epilogue tile time-shares the psv bank): exp(s)
# frees its slot two swaths before the PE needs it again, so the score
# stream no longer serializes behind in-flight exps (with two slots the
# steady state was chain-paced at exp+scores+handoff per 2 swaths)
SW = [3, 2, 2] * 4 + [2, 2]
# swath indices whose exp runs on the DVE (rest on ACT). All 3-chunk slots
# (si%3==0, the longest chain links) go to the faster ACT engine; 2-chunk
# tiles mostly go to DVE so both engines stay balanced and concurrent
DVE_SW = {1, 2, 5, 7, 8, 11, 13}
# chunks computed as one full-array K=128 zero-padded matmul instead of a
# K=64 2x2 grid pair: one full matmul every ~1.7us keeps the HAM clock gate
# at 2.4 GHz (partial-array matmuls alone read as "idle" and it re-throttles;
# FULL_EVERY=6 measured marginal against the 3.4us MID window -- roughly half
# of runs re-throttled mid-kernel and ran the PE cold)
FULL_EVERY = 4
FULL_CHUNKS = tuple(range(0, MC, FULL_EVERY))


def _register_exp16():
    """Custom DVE op: out = (1 + z/16 + (z/16)^2/2)^16 ~ exp(z).

    Registered via the documented dve_ops extension path (append to OPS);
    done at import time here because the concourse tree is read-only.
    8 ALU stages: mul, add, mul, add, 4x square -> 1 elem/lane/cycle.
    """
    import concourse.dve_ops as dve_ops
    from concourse.dve_spec import Spec, Src0, One, C1, C2, sq

    if "EXP16_SAGAN" in dve_ops._SUB_OPCODE_FOR_NAME:
        return next(o for o in dve_ops.OPS if o.name == "EXP16_SAGAN")

    def _ref(in0, in1, s0, s1, imm2):
        z = in0.astype(np.float32)
        q = (1.0 + z * np.float32(imm2) + (z * np.float32(imm2)) ** 2 / 2.0)
        return (q ** 16).astype(np.float32)

    t2 = Src0 * C1 + C2
    q = Src0 * t2 + One
    op = dve_ops.DveOp(
        "EXP16_SAGAN",
        Spec(body=sq(sq(sq(sq(q)))), reference=_ref),
        subdim=False,
        uops_sha={"v3": "b6b488f5e7e070ba", "v4": "cea1bf697c4845eb"},
    )
    dve_ops.OPS.append(op)
    dve_ops.CUSTOM_DVE_SPECS[op.name] = op.spec
    dve_ops._SUB_OPCODE_FOR_NAME[op.name] = (
        dve_ops._CUSTOM_DVE_ROW_BASE + len(dve_ops.OPS) - 1
    )
    return op


EXP16_OP = _register_exp16()


def _build():
    nc = bacc.Bacc("TRN2", target_bir_lowering=False, debug=False,
                   num_devices=NCORES)

    xt2 = nc.dram_tensor("xt2", [128, N], BF16, kind="ExternalInput").ap()
    xt0 = nc.dram_tensor("xt0", [128, len(FULL_CHUNKS) * 128], BF16,
                         kind="ExternalInput").ap()
    gp2 = nc.dram_tensor("gp2", [128, RPC], BF16, kind="ExternalInput").ap()
    hvo = nc.dram_tensor("hvo", [128, MC * 32], BF16,
                         kind="ExternalInput").ap()
    xrp = nc.dram_tensor("xrp", [128, RPC // 128 * C], F32,
                         kind="ExternalInput").ap()
    wv2 = nc.dram_tensor("wv2", [128, C + 1], BF16, kind="ExternalInput").ap()
    out = nc.dram_tensor("out", [128, RPC // 128 * C], F32,
                         kind="ExternalOutput").ap()

    with tile.TileContext(nc) as tc:
        with tc.tile_pool(name="const", bufs=1) as const:
            XT2 = const.tile([128, N], BF16)
            XT0 = const.tile([128, len(FULL_CHUNKS) * 128], BF16)
            GP2 = const.tile([128, RPC], BF16)
            HVO = const.tile([128, MC * 32], BF16)
            XRP = const.tile([128, RPC // 128 * C], F32)
            WVE = const.tile([128, C + 1], BF16)
            PRE = const.tile([1, 1], F32)
            PRE2 = const.tile([1, 4], BF16)
            WUP = const.tile([128, 256], BF16)

            # input DMAs in first-use order: swath 0 of n-tile 0 needs
            # GP2 cols 0:512, XT2 cols 0:384, HVO cols 0:96
            nc.sync.dma_start(GP2[:, 0:512], gp2[:, 0:512])
            nc.sync.dma_start(XT2[:, 0:512], xt2[:, 0:512])
            nc.sync.dma_start(HVO[:, 0:256], hvo[:, 0:256])
            nc.sync.dma_start(XT2[:, 512:2048], xt2[:, 512:2048])
            nc.sync.dma_start(HVO[:, 256:1024], hvo[:, 256:1024])
            nc.sync.dma_start(XT2[:, 2048:4096], xt2[:, 2048:4096])
            nc.sync.dma_start(XT0[:], xt0[:])
            nc.sync.dma_start(GP2[:, 512:2048], gp2[:, 512:2048])
            nc.sync.dma_start(WVE[:], wv2[:])
            nc.sync.dma_start(XRP[:], xrp[:])
            nc.vector.memset(WUP[:], 0.0)
            # dummy exp: hoists the one-time ACT table load (~2.7us) into the
            # initial DMA wait instead of the first swath's critical path
            nc.scalar.activation(PRE[:], WUP[0:1, 0:1], AFT.Exp)
            # dummy EXP16: flushes any lazy DVE custom-table init
            nc.vector._custom_dve(EXP16_OP, out=PRE2[:, 0:1],
                                  in0=WUP[0:1, 0:1],
                                  s1=1.0 / 512, imm2=1.0 / 16)

            with tc.tile_pool(name="ps_a", bufs=1, space="PSUM") as ps_a, \
                 tc.tile_pool(name="ps_b", bufs=1, space="PSUM") as ps_b, \
                 tc.tile_pool(name="ps_c", bufs=1, space="PSUM") as ps_c, \
                 tc.tile_pool(name="ps_v", bufs=1, space="PSUM") as ps_vp, \
                 tc.tile_pool(name="expp", bufs=8) as expp, \
                 tc.tile_pool(name="vd2p", bufs=2) as vd2p, \
                 tc.tile_pool(name="scolp", bufs=2) as scolp, \
                 tc.tile_pool(name="osbp", bufs=2) as osbp:
                # PE warm-up: ~4us of K=128 matmuls during the initial DMA
                # wait lifts the HAM clock throttle (1.2 -> 2.4 GHz) before
                # the real swaths begin; output is scratch, never read
                wps = ps_a.tile([128, 1536], F32, tag="ps")
                for wi in range(14):
                    nc.tensor.matmul(wps[:, 0:256], lhsT=WUP[:, 0:128],
                                     rhs=WUP[:], start=True, stop=True,
                                     skip_group_check=True)

                def epilogue(nt, vd2, split=False):
                    # one [128,65] rhs per query block: cols 0-63 -> o_un,
                    # col 64 (E8 one-hots) -> summed denominator. split=True
                    # (final n-tile) pipelines two halves to shorten the
                    # tail. The tile time-shares the psv bank: psv(nt) was
                    # fully read (vd2 copy) before this, and psv(nt+1) is
                    # allocated lazily after this tile's STT reads finish
                    pse_t = ps_vp.tile([128, 512], F32, tag="psv")
                    pse = pse_t[:, 0:260]
                    scol = scolp.tile([128, 4], F32)
                    osb4 = osbp.tile([128, 4 * C], F32)
                    for h in range(2) if split else (0,):
                        nbs = (2 * h, 2 * h + 1) if split else range(4)
                        for nb in nbs:
                            nc.tensor.matmul(
                                pse[:, nb * 65:(nb + 1) * 65],
                                lhsT=vd2[:, nb * 128:(nb + 1) * 128],
                                rhs=WVE[:], start=True, stop=True)
                        if split:
                            nc.vector.reciprocal(
                                scol[:, 2 * h:2 * h + 2],
                                pse[:, 130 * h + 64:130 * h + 130:65])
                        else:
                            nc.vector.reciprocal(scol[:], pse[:, 64:260:65])
                        for nb in nbs:
                            t = nt * 4 + nb
                            nc.vector.scalar_tensor_tensor(
                                osb4[:, nb * C:(nb + 1) * C],
                                pse[:, nb * 65:nb * 65 + 64],
                                scol[:, nb:nb + 1],
                                XRP[:, t * C:(t + 1) * C],
                                op0=AluOpType.mult, op1=AluOpType.add)
                        if split:
                            o0 = nt * 4 * C + h * 2 * C
                            nc.sync.dma_start(out[:, o0:o0 + 2 * C],
                                              osb4[:, h * 2 * C:(h + 1) * 2 * C])
                    if not split:
                        nc.sync.dma_start(out[:, nt * 4 * C:(nt + 1) * 4 * C],
                                          osb4[:])

                def emit_pv(psv, ex, m0, sw):
                    for k in range(sw):
                        mm = m0 + k
                        c4 = (mm % 4) * 32
                        nc.tensor.matmul(
                            psv[c4:c4 + 32, :],
                            lhsT=HVO[:, mm * 32:(mm + 1) * 32],
                            rhs=ex[:, k * 512:(k + 1) * 512],
                            start=(mm < 4), stop=(mm >= MC - 4),
                            skip_group_check=True,
                            tile_position=(0, c4))

                pending = None
                for nt in range(NT):
                    n0 = nt * TN
                    psv = None   # allocated lazily AFTER the epilogue's pse
                    m = 0        # so the shared ps_v pool rotates in order
                    prev_pv = []
                    for si, sw in enumerate(SW):
                        pool = (ps_a, ps_b, ps_c)[si % 3]
                        ps = pool.tile([128, 1536 if si % 3 == 0 else 1024],
                                       F32)
                        ex = expp.tile([128, 1536], BF16)
                        w = sw * 512
                        # the first swaths run every chunk as a full-array
                        # K=128 matmul on the duplicated channel rows (scores
                        # come out doubled; the exp scale of 0.5 corrects) so
                        # the HAM clock gate warms through the startup ramp.
                        # Everything after uses K=64 grid pairs with a
                        # full-array XT0 matmul every FULL_EVERY chunks as
                        # heartbeat.
                        dup = nt == 0 and si < 5
                        for k in range(sw):
                            mm = m + k
                            if dup or mm % FULL_EVERY == 0:
                                if dup:
                                    lw = XT2[:, mm * 128:(mm + 1) * 128]
                                else:
                                    fi = mm // FULL_EVERY
                                    lw = XT0[:, fi * 128:(fi + 1) * 128]
                                nc.tensor.matmul(
                                    ps[:, k * 512:(k + 1) * 512],
                                    lhsT=lw, rhs=GP2[:, n0:n0 + TN],
                                    start=True, stop=True)
                                continue
                            cA = mm * 128
                            # K=64 grid pair on disjoint array quadrants;
                            # parity alternates the channel-copy row strips
                            rl = 0 if mm % 2 == 0 else 64
                            rh = 64 - rl
                            nc.tensor.matmul(
                                ps[0:64, k * 512:(k + 1) * 512],
                                lhsT=XT2[rl:rl + 64, cA:cA + 64],
                                rhs=GP2[rl:rl + 64, n0:n0 + TN],
                                start=True, stop=True)
                            nc.tensor.matmul(
                                ps[64:128, k * 512:(k + 1) * 512],
                                lhsT=XT2[rh:rh + 64, cA + 64:cA + 128],
                                rhs=GP2[rh:rh + 64, n0:n0 + TN],
                                start=True, stop=True)
                        sc = 0.5 if dup else 1.0
                        if si in DVE_SW:
                            nc.vector._custom_dve(
                                EXP16_OP, out=ex[:, :w], in0=ps[:, :w],
                                s1=sc * sc / 512, imm2=sc / 16)
                        else:
                            nc.scalar.activation(ex[:, :w], ps[:, :w],
                                                 AFT.Exp, scale=sc)
                        # PV is emitted one-to-two swaths late in PAIRS of
                        # swath groups: the PE queue always has score matmuls
                        # during exp, and 5-6 back-to-back col-tiled PV
                        # matmuls amortize one fill+drain (same-strip psum
                        # accumulates are hw-serialized in order and commute)
                        prev_pv.append((ex, m, sw))
                        if len(prev_pv) == 3:
                            if psv is None:
                                psv = ps_vp.tile([128, TN], F32,
                                                 tag="psv")
                            for args in prev_pv[:2]:
                                emit_pv(psv, *args)
                            prev_pv = prev_pv[2:]
                        m += sw
                        # previous n-tile's epilogue in swath 0: its DVE work
                        # (recip+STT) lands in the DVE queue during the
                        # n-tile-boundary refill slack instead of between
                        # chain-critical exp calls
                        if si == 0 and pending is not None:
                            epilogue(*pending)
                            pending = None
                    for args in prev_pv:
                        emit_pv(psv, *args)
                    vd2 = vd2p.tile([128, TN], BF16)
                    # cast split across BOTH engines: halves the insertion
                    # each exp queue takes at the n-tile boundary, and lets
                    # the tail epilogue's first half start earlier
                    nc.scalar.copy(vd2[:, 0:256], psv[:, 0:256])
                    nc.vector.tensor_copy(vd2[:, 256:512], psv[:, 256:512])
                    pending = (nt, vd2)
                epilogue(*pending, split=True)
    nc.compile()
    return nc


_CACHE = {}


def _get_compiled():
    if "nc" not in _CACHE:
        _CACHE["nc"] = _build()
    return _CACHE["nc"]


def _make_in_maps(x, Wf, bf, Wg, bg, Wh, bh, Wv, bv, gamma):
    x = np.asarray(x, np.float32)
    Wf = np.asarray(Wf, np.float32)
    Wg = np.asarray(Wg, np.float32)
    Wh = np.asarray(Wh, np.float32)
    Wv = np.asarray(Wv, np.float32)
    bf = np.asarray(bf, np.float32)
    bg = np.asarray(bg, np.float32)
    bh = np.asarray(bh, np.float32)
    bv = np.asarray(bv, np.float32)
    g0 = float(np.asarray(gamma, np.float32).reshape(-1)[0])

    xf = x.reshape(B, N, C)
    P = Wf @ Wg.T                            # [C, C] score kernel
    wfbg = Wf @ bg                           # [C] per-key bias direction
    bgbf = float(bg @ bf)
    res_bias = g0 * (bh @ Wv + bv)           # [C] folded into residual
    wv2 = np.zeros((128, C + 1), np.float32)
    for sb in range(4):                      # strip-summed by the epilogue MM
        wv2[sb * 32 + 1:sb * 32 + 1 + D, 0:C] = g0 * Wv
        wv2[sb * 32, C] = 1.0                # E8 one-hot -> denominator sum
    wv2 = wv2.astype(ml_dtypes.bfloat16)

    in_maps = []
    for i in range(NCORES):
        b, h = divmod(i, 2)
        r0 = h * RPC
        xt2 = np.empty((128, N), np.float32)
        xt2[0:C] = xf[b].T                   # channels, duplicated per half
        xt2[C:2 * C] = xf[b].T               # (for the K=64 grid pairs)
        gp2 = np.empty((128, RPC), np.float32)
        gp2[0:C] = P @ xf[b, r0:r0 + RPC].T
        gp2[C:2 * C] = gp2[0:C]
        xt0 = np.zeros((128, len(FULL_CHUNKS) * 128), np.float32)
        for fi, mm in enumerate(FULL_CHUNKS):  # zero rows 64-127: full-K
            xt0[0:C, fi * 128:(fi + 1) * 128] = (  # matmuls ignore the dup
                xf[b, mm * 128:(mm + 1) * 128].T)
        d_m = xf[b] @ wfbg + bgbf            # per-key score bias
        ed = np.exp(d_m)                     # folded multiplicatively
        hv = xf[b] @ Wh                      # [N, D] (bh folds into res_bias)
        ho = np.zeros((MC, 128, 32), np.float32)
        ho[:, :, 0] = ed.reshape(MC, 128)
        ho[:, :, 1:1 + D] = (ed[:, None] * hv).reshape(MC, 128, D)
        ho = np.ascontiguousarray(ho.transpose(1, 0, 2).reshape(128, -1))
        xr = xf[b, r0:r0 + RPC] + res_bias   # [RPC, C]
        xrp = np.ascontiguousarray(
            xr.reshape(RPC // 128, 128, C).transpose(1, 0, 2).reshape(128, -1))
        in_maps.append({"xt2": xt2.astype(ml_dtypes.bfloat16),
                        "xt0": xt0.astype(ml_dtypes.bfloat16),
                        "gp2": gp2.astype(ml_dtypes.bfloat16),
                        "hvo": ho.astype(ml_dtypes.bfloat16),
                        "xrp": xrp, "wv2": wv2})
    return in_maps


def _assemble(results):
    outf = np.empty((B, N, C), np.float32)
    for i in range(NCORES):
        b, h = divmod(i, 2)
        o = results[i]["out"].reshape(128, RPC // 128, C)
        outf[b, h * RPC:(h + 1) * RPC] = (
            o.transpose(1, 0, 2).reshape(RPC, C))
    return outf.reshape(B, HH, WW, C)


def run(inputs, **spmd_kwargs):
    """Returns (output, BassKernelResults)."""
    nc = _get_compiled()
    in_maps = _make_in_maps(**inputs)
    res = run_bass_kernel_spmd(nc, in_maps, core_ids=list(range(NCORES)),
                               **spmd_kwargs)
    return _assemble(res.results), res


def kernel(**inputs):
    out, _ = run(inputs)
    return out
